# revision 30
# baseline (speedup 1.0000x reference)
"""2D Gaussian splat rasterizer on 8 Trainium2 NeuronCores.

Strategy: shard the image into 128 tiles of 16x32 px (F=512), dealt to 8
cores (16 slots each). Gaussians are culled host-side per tile by the
EXACT min-Mahalanobis-over-rect test (<= raster_ratio^2) and packed into
chunks of <=128 (partition dim). Per chunk, on device:

    arg   = coefT.T @ basis      TensorE, K=12 fp16, 1 cy/row: the 6-term
                                 pixel-basis [x2, xy, y2, x, y, 1] quadratic
                                 (tile-local coords) with each coefficient
                                 split hi/lo across two fp16 rows (full-f32
                                 effective precision; basis values are all
                                 exactly representable in fp16), constant
                                 row folds in ln(opacity)
    alpha = Exp(arg)             ScalarE from PSUM, fp16 out, fused over
                                 FUSE consecutive chunks (no bias, no mask:
                                 the cutoff is applied by the exact cull,
                                 in-tile tails are within tolerance)
    out  += colors.T @ alpha     TensorE, K=128 fp16, PSUM accumulate;
                                 4 tiles share one PSUM bank via
                                 tile_position column groups {0,32,64,96}

Per round of 4 tiles one DVE copy moves the PSUM bank to fp16 SBUF; one
final DMA (partition-strided AP) writes all 16 tiles out. 3 DMAs total.
The full [H, W, 3] image is reassembled host-side (no collectives).
"""

import numpy as np
import concourse.bacc as bacc
import concourse.tile as tile
from concourse import mybir
from concourse.bass_utils import run_bass_kernel_spmd

_runner_cache = {}


def _get_runner(nc):
    """Persistent jitted SPMD executor for a compiled Bass program (modeled on
    bass2jax.run_bass_via_pjrt's multi-core path, but cached so repeat calls
    reuse the same XLA executable — no retrace, no NEFF reload)."""
    key = id(nc)
    if key in _runner_cache:
        return _runner_cache[key]
    import jax
    import jax.numpy as jnp
    from jax.sharding import Mesh, PartitionSpec
    from jax.experimental.shard_map import shard_map
    from concourse import bass2jax, mybir as mb

    bass2jax.install_neuronx_cc_hook()

    in_names, out_names, out_avals, zero_outs = [], [], [], []
    partition_name = nc.partition_id_tensor.name if nc.partition_id_tensor else None
    for alloc in nc.m.functions[0].allocations:
        if not isinstance(alloc, mb.MemoryLocationSet):
            continue
        name = alloc.memorylocations[0].name
        if alloc.kind == "ExternalInput":
            if name != partition_name:
                in_names.append(name)
        elif alloc.kind == "ExternalOutput":
            shape = tuple(alloc.tensor_shape)
            dtype = mb.dt.np(alloc.dtype)
            out_names.append(name)
            out_avals.append(jax.core.ShapedArray(shape, dtype))
            zero_outs.append(np.zeros(shape, dtype))
    n_params = len(in_names)
    all_in = in_names + out_names + ([partition_name] if partition_name else [])

    def _body(*args):
        operands = list(args)
        if partition_name is not None:
            operands.append(bass2jax.partition_id_tensor())
        outs = bass2jax._bass_exec_p.bind(
            *operands,
            out_avals=tuple(out_avals),
            in_names=tuple(all_in),
            out_names=tuple(out_names),
            lowering_input_output_aliases=(),
            sim_require_finite=True,
            sim_require_nnan=True,
            nc=nc,
        )
        return tuple(outs)

    devices = jax.devices()[:N_CORES]
    mesh = Mesh(np.asarray(devices), ("core",))
    in_specs = (PartitionSpec("core"),) * (n_params + len(out_names))
    out_specs = (PartitionSpec("core"),) * len(out_names)
    sharded = jax.jit(
        shard_map(
            _body, mesh=mesh, in_specs=in_specs, out_specs=out_specs, check_rep=False
        ),
        donate_argnums=tuple(range(n_params, n_params + len(out_names))),
        keep_unused=True,
    )

    dev_in_cache = {}

    def run(in_maps, reuse_inputs=False):
        if reuse_inputs and "in" in dev_in_cache:
            concat_in = dev_in_cache["in"]
        else:
            concat_in = [
                np.concatenate([np.asarray(m[nm]) for m in in_maps], axis=0)
                for nm in in_names
            ]
            if reuse_inputs:
                from jax.sharding import NamedSharding

                sh = NamedSharding(mesh, PartitionSpec("core"))
                concat_in = [jax.device_put(a, sh) for a in concat_in]
                for a in concat_in:
                    a.block_until_ready()
                dev_in_cache["in"] = concat_in
        concat_zeros = [
            np.zeros((N_CORES * z.shape[0], *z.shape[1:]), z.dtype) for z in zero_outs
        ]
        out_arrs = sharded(*concat_in, *concat_zeros)
        out_arrs = [a.block_until_ready() for a in out_arrs]
        return [
            {
                nm: np.asarray(out_arrs[i]).reshape(N_CORES, *out_avals[i].shape)[c]
                for i, nm in enumerate(out_names)
            }
            for c in range(N_CORES)
        ]

    def time_loop(in_maps, n_calls):
        """Per-call wall times with inputs and donated zero-outputs pre-staged
        on device; outputs stay on device (only block_until_ready)."""
        import time as _t
        from jax.sharding import NamedSharding

        sh = NamedSharding(mesh, PartitionSpec("core"))
        concat_in = [
            jax.device_put(
                np.concatenate([np.asarray(m[nm]) for m in in_maps], axis=0), sh
            )
            for nm in in_names
        ]
        zeros_sets = [
            [
                jax.device_put(
                    np.zeros((N_CORES * z.shape[0], *z.shape[1:]), z.dtype), sh
                )
                for z in zero_outs
            ]
            for _ in range(n_calls)
        ]
        for a in concat_in:
            a.block_until_ready()
        for zs in zeros_sets:
            for a in zs:
                a.block_until_ready()
        # warm once (executable load)
        outs = sharded(*concat_in, *zeros_sets[0])
        [a.block_until_ready() for a in outs]
        times = []
        for i in range(1, n_calls):
            t0 = _t.perf_counter()
            outs = sharded(*concat_in, *zeros_sets[i])
            [a.block_until_ready() for a in outs]
            times.append(_t.perf_counter() - t0)
        return times

    def stage(in_maps, n_calls):
        """Pre-stage inputs + n_calls sets of donated zeros; return a closure
        that executes once per call (device exec + block)."""
        from jax.sharding import NamedSharding

        sh = NamedSharding(mesh, PartitionSpec("core"))
        concat_in = [
            jax.device_put(
                np.concatenate([np.asarray(m[nm]) for m in in_maps], axis=0), sh
            )
            for nm in in_names
        ]
        zeros_sets = [
            [
                jax.device_put(
                    np.zeros((N_CORES * z.shape[0], *z.shape[1:]), z.dtype), sh
                )
                for z in zero_outs
            ]
            for _ in range(n_calls)
        ]
        for a in concat_in:
            a.block_until_ready()
        for zs in zeros_sets:
            for a in zs:
                a.block_until_ready()
        state = {"i": 0}

        def call():
            i = state["i"]
            state["i"] += 1
            outs = sharded(*concat_in, *zeros_sets[i])
            # force full materialization — under the axon proxy,
            # block_until_ready alone does not wait for device execution
            return [np.asarray(a) for a in outs]

        return call

    def stage_async(in_maps, n_calls):
        """Like stage() but returns call(block=False) that does not wait."""
        from jax.sharding import NamedSharding

        sh = NamedSharding(mesh, PartitionSpec("core"))
        concat_in = [
            jax.device_put(
                np.concatenate([np.asarray(m[nm]) for m in in_maps], axis=0), sh
            )
            for nm in in_names
        ]
        zeros_sets = [
            [
                jax.device_put(
                    np.zeros((N_CORES * z.shape[0], *z.shape[1:]), z.dtype), sh
                )
                for z in zero_outs
            ]
            for _ in range(n_calls)
        ]
        for a in concat_in:
            a.block_until_ready()
        for zs in zeros_sets:
            for a in zs:
                a.block_until_ready()
        state = {"i": 0}

        def call(block=False):
            i = state["i"]
            state["i"] += 1
            outs = sharded(*concat_in, *zeros_sets[i])
            if block:
                outs = [np.asarray(a) for a in outs]
            return outs

        return call

    run.time_loop = time_loop
    run.stage = stage
    run.stage_async = stage_async
    _runner_cache[key] = run
    return run


N_CORES = 8
K = 12  # 6 basis terms x (hi, lo) coefficient rows
TILE_ROWS = 16
TILE_COLS = 32
F = TILE_ROWS * TILE_COLS  # 512 pixels per tile
QUADS = 1  # PE row-group rotation for arg matmul weight loads
FUSE = 3  # chunks per fused Exp activation
GROUP_TILES = 4  # tiles sharing one PSUM out bank via column groups

_prog_cache = {}


FUSE_COLS = 3 * F  # fused-group PSUM capacity (3 banks)


def _build_program(slot_profiles, cutoff, repeat=1):
    """One SPMD program: 16 slots (tiles) per core; slot_profiles[s] is a
    tuple of (c0, w) chunk descriptors — each chunk covers basis/pixel
    columns [c0, c0+w) of its tile and up to 128 gaussians.

    cutoff is unused (kept for cache-key/test harness compatibility)."""
    n_slots = len(slot_profiles)
    n_rounds = n_slots // GROUP_TILES
    tot = sum(len(p) for p in slot_profiles)
    X = tot * 128  # coef columns
    CB = X + F  # coef | basis columns
    nc = bacc.Bacc(
        "TRN2",
        target_bir_lowering=False,
        debug=False,
        enable_asserts=True,
        num_devices=N_CORES,
    )
    f32, f16 = mybir.dt.float32, mybir.dt.float16
    cb_ext = nc.dram_tensor("cb", [QUADS, K, CB], f16, kind="ExternalInput").ap()
    colors_ext = nc.dram_tensor("colors", [128, tot * 3], f16, kind="ExternalInput").ap()
    # 99 partition lines: 4 tile groups at partition offsets {0,32,64,96},
    # rows 3..31 of each group are don't-care (host reads rows 32*i..32*i+2)
    out_ext = nc.dram_tensor("out", [99, n_rounds * F], f16,
                             kind="ExternalOutput").ap()

    # per-round chunk lists: (slot, c0, w, first, last, global idx)
    j = 0
    round_flat = []
    for r in range(n_rounds):
        fl = []
        for s in range(r * GROUP_TILES, (r + 1) * GROUP_TILES):
            prof = slot_profiles[s]
            seen = {}
            for (c0, w) in prof:
                seen[c0] = seen.get(c0, 0) + 1
            done = {}
            for (c0, w) in prof:
                done[c0] = done.get(c0, 0) + 1
                fl.append((s, c0, w, done[c0] == 1, done[c0] == seen[c0], j))
                j += 1
        round_flat.append(fl)

    # greedy fused groups per round: pack chunks into <=FUSE_COLS of PSUM,
    # 512-wide chunks only at 512-aligned offsets (PSUM bank writes)
    def make_groups(fl):
        groups = []
        cur, off = [], 0
        for ch in fl:
            w = ch[2]
            if off + w > FUSE_COLS or (w == F and off % F != 0):
                groups.append(cur)
                cur, off = [], 0
            cur.append((ch, off))
            off += w
        if cur:
            groups.append(cur)
        return groups

    with tile.TileContext(nc) as tc:
        with (
            tc.tile_pool(name="consts", bufs=1) as consts,
            tc.tile_pool(name="work", bufs=3) as work,
            tc.tile_pool(name="outsb", bufs=1) as outsb,
            tc.tile_pool(name="psum_arg", bufs=2, space="PSUM") as psum_arg,
            tc.tile_pool(name="psum_out", bufs=2, space="PSUM") as psum_out,
        ):
            cb_sb = consts.tile([128, CB], f16, name="cb_sb")
            for q in range(QUADS):
                nc.sync.dma_start(
                    out=cb_sb[32 * q : 32 * q + K, :], in_=cb_ext[q]
                )
            colors_sb = consts.tile([128, tot * 3], f16, name="colors_sb")
            nc.sync.dma_start(out=colors_sb[:], in_=colors_ext[:])
            out_sb = outsb.tile([128, n_rounds * F], f16, name="out_sb")

            # preload the Exp activation table while the input DMAs are in
            # flight (the table load rides on this first tiny activation)
            warm_sb = consts.tile([1, 8], f16, name="warm_sb")
            nc.vector.memset(warm_sb[:], 0.0)
            nc.scalar.activation(
                warm_sb[:], warm_sb[:], mybir.ActivationFunctionType.Exp
            )

            for r in range(n_rounds):
                out_ps = psum_out.tile([99, F], f32, tag="out", name=f"out_ps_{r}")
                groups = make_groups(round_flat[r])
                for rep in range(repeat):
                    for g in groups:
                        used = sum(ch[2] for ch, _ in g)
                        arg_ps = psum_arg.tile([128, FUSE_COLS], f32, tag="arg")
                        for (s, c0, w, first, last, jj), off in g:
                            p0 = 32 * (jj % QUADS)
                            nc.tensor.matmul(
                                arg_ps[:, off : off + w],
                                lhsT=cb_sb[p0 : p0 + K, jj * 128 : (jj + 1) * 128],
                                rhs=cb_sb[p0 : p0 + K, X + c0 : X + c0 + w],
                                start=True,
                                stop=True,
                                tile_position=(p0, 0),
                            )
                        alpha_sb = work.tile([128, FUSE_COLS], f16, tag="alpha")
                        nc.scalar.activation(
                            alpha_sb[:, 0:used],
                            arg_ps[:, 0:used],
                            mybir.ActivationFunctionType.Exp,
                        )
                        for (s, c0, w, first, last, jj), off in g:
                            p0 = 32 * (s % GROUP_TILES)
                            nc.tensor.matmul(
                                out_ps[p0 : p0 + 3, c0 : c0 + w],
                                lhsT=colors_sb[:, jj * 3 : (jj + 1) * 3],
                                rhs=alpha_sb[:, off : off + w],
                                start=(first and rep == 0),
                                stop=(last and rep == repeat - 1),
                                tile_position=(0, p0),
                            )
                nc.vector.tensor_copy(out_sb[0:99, r * F : (r + 1) * F], out_ps[:])
                nc.sync.dma_start(
                    out=out_ext[:, r * F : (r + 1) * F],
                    in_=out_sb[0:99, r * F : (r + 1) * F],
                )
    nc.compile()
    return nc


def _get_program(slot_nch, cutoff, repeat=1):
    key = (tuple(slot_nch), float(cutoff), repeat)
    if key not in _prog_cache:
        _prog_cache[key] = _build_program(slot_nch, cutoff, repeat)
    return _prog_cache[key]


def _basis():
    """[6, F] f64 basis terms in tile-local coords (all fp16-exact).

    Pixel-column order: left half-tile (16x16, row-major) then right half,
    so half-width chunks stream contiguous column ranges."""
    hw = TILE_COLS // 2
    cols = np.concatenate(
        [np.arange(hw), np.arange(hw, TILE_COLS)]
    )  # identity, kept for clarity
    xs = np.arange(TILE_COLS, dtype=np.float64) + 0.5 - TILE_COLS / 2
    ys = np.arange(TILE_ROWS, dtype=np.float64) + 0.5 - TILE_ROWS / 2
    xl = np.empty(F)
    yl = np.empty(F)
    for h in range(2):
        for row in range(TILE_ROWS):
            for col in range(hw):
                p = h * (F // 2) + row * hw + col
                xl[p] = xs[h * hw + col]
                yl[p] = ys[row]
    return np.stack([xl * xl, xl * yl, yl * yl, xl, yl, np.ones_like(xl)], axis=0)


def kernel(
    opacity,
    means,
    stds,
    rhos,
    colors,
    image_height,
    image_width,
    scale_factor,
    raster_ratio,
    _repeat=1,
    _time_exec=False,
    _bench_calls=0,
):
    H = int(image_height)
    W = int(image_width)
    sf = float(scale_factor)
    rr = float(raster_ratio)
    opacity = np.asarray(opacity, np.float64)
    means = np.asarray(means, np.float64)
    stds = np.asarray(stds, np.float64) * sf
    rhos = np.asarray(rhos, np.float64)
    colors = np.asarray(colors, np.float32)
    N = opacity.shape[0]

    n_ty = H // TILE_ROWS
    n_tx = W // TILE_COLS
    n_tiles = n_ty * n_tx
    assert n_tiles % N_CORES == 0
    n_slots = n_tiles // N_CORES
    assert n_slots % GROUP_TILES == 0

    # --- per-gaussian inverse covariance (f64)
    sx, sy = stds[:, 0], stds[:, 1]
    om = 1.0 - rhos * rhos
    ia = 1.0 / (sx * sx * om)
    ib = -rhos / (sx * sy * om)
    ic = 1.0 / (sy * sy * om)
    mx, my = means[:, 0], means[:, 1]
    lnop = np.log(np.maximum(opacity, 1e-30))

    # --- exact ellipse-vs-rect cull: min Mahalanobis^2 over pixel centers
    cut2 = rr * rr + 1e-6

    def min_m2(x0, x1, y0, y1):
        dx0, dx1 = x0 - mx, x1 - mx
        dy0, dy1 = y0 - my, y1 - my
        inside = (dx0 <= 0) & (dx1 >= 0) & (dy0 <= 0) & (dy1 >= 0)
        best = np.full(N, np.inf)
        for cdx in (dx0, dx1):
            dy = np.clip(-ib * cdx / ic, dy0, dy1)
            best = np.minimum(best, ia * cdx * cdx + 2 * ib * cdx * dy + ic * dy * dy)
        for cdy in (dy0, dy1):
            dx = np.clip(-ib * cdy / ia, dx0, dx1)
            best = np.minimum(best, ia * dx * dx + 2 * ib * cdy * dx + ic * cdy * cdy)
        return np.where(inside, 0.0, best)

    hw2 = TILE_COLS // 2
    tile_ids = []  # full-tile cull
    tile_ids_h = []  # (left, right) half-tile culls
    tile_pos = []
    for tyi in range(n_ty):
        ty = tyi * TILE_ROWS
        y0, y1 = ty + 0.5, ty + TILE_ROWS - 0.5
        for txi in range(n_tx):
            tx = txi * TILE_COLS
            m2 = min_m2(tx + 0.5, tx + TILE_COLS - 0.5, y0, y1)
            m2l = min_m2(tx + 0.5, tx + hw2 - 0.5, y0, y1)
            m2r = min_m2(tx + hw2 + 0.5, tx + TILE_COLS - 0.5, y0, y1)
            tile_ids.append(np.nonzero(m2 <= cut2)[0])
            tile_ids_h.append(
                (np.nonzero(m2l <= cut2)[0], np.nonzero(m2r <= cut2)[0])
            )
            tile_pos.append((ty, tx))

    # snake-deal tiles to cores by descending chunk need so the SPMD slot
    # capacities (max over cores per slot) hug each core's real need
    nchs = [max(1, (len(ids) + 127) // 128) for ids in tile_ids]
    t_order = sorted(range(n_tiles), key=lambda t: -nchs[t])
    assign = [[] for _ in range(N_CORES)]
    for i, t in enumerate(t_order):
        rnd, pos = divmod(i, N_CORES)
        core = pos if rnd % 2 == 0 else N_CORES - 1 - pos
        assign[core].append(t)
    # permute slots so round chunk-counts hug multiples of FUSE (heavy slots
    # paired with light slots within a round)
    perm = []
    lo, hi = 0, n_slots - 1
    while lo < hi:
        perm.extend([lo, lo + 1, hi - 1, hi])
        lo += 2
        hi -= 2
    assign = [[a[p] for p in perm] for a in assign]
    # slot profiles: single full-width chunk when every core's tile fits 128
    # gaussians; otherwise half-width chunks with per-half culls
    slot_profiles = []
    for k in range(n_slots):
        tiles_k = [assign[core][k] for core in range(N_CORES)]
        if max(nchs[t] for t in tiles_k) == 1:
            slot_profiles.append(((0, F),))
        else:
            aL = max(1, max((len(tile_ids_h[t][0]) + 127) // 128 for t in tiles_k))
            aR = max(1, max((len(tile_ids_h[t][1]) + 127) // 128 for t in tiles_k))
            slot_profiles.append(
                ((0, F // 2),) * aL + ((F // 2, F // 2),) * aR
            )
    slot_profiles = tuple(slot_profiles)
    tot = sum(len(p) for p in slot_profiles)
    X = tot * 128

    nc = _get_program(slot_profiles, 0.0, _repeat)

    basis6 = _basis()  # [6, F] f64, fp16-exact values

    def coef_hi_lo(ids, cxo, cyo):
        mxl = mx[ids] - cxo
        myl = my[ids] - cyo
        A, B, C = ia[ids], ib[ids], ic[ids]
        cf = np.stack(
            [
                -0.5 * A,
                -B,
                -0.5 * C,
                A * mxl + B * myl,
                B * mxl + C * myl,
                -0.5 * (A * mxl * mxl + 2 * B * mxl * myl + C * myl * myl)
                + lnop[ids],
            ],
            axis=0,
        )  # [6, n] f64
        hi = cf.astype(np.float16)
        lo = (cf - hi.astype(np.float64)).astype(np.float16)
        return hi, lo

    in_maps = []
    for core in range(N_CORES):
        cb_arr = np.zeros((QUADS, K, X + F), np.float16)
        colors_arr = np.zeros((128, tot * 3), np.float16)
        jj = 0
        for k in range(n_slots):
            t = assign[core][k]
            ty, tx = tile_pos[t]
            cxo = tx + TILE_COLS / 2
            cyo = ty + TILE_ROWS / 2
            used = {}  # c0 -> chunks consumed for that side
            for (c0, w) in slot_profiles[k]:
                if w == F:
                    ids_side = tile_ids[t]
                else:
                    ids_side = tile_ids_h[t][0 if c0 == 0 else 1]
                c = used.get(c0, 0)
                used[c0] = c + 1
                ids = ids_side[c * 128 : (c + 1) * 128]
                gn = len(ids)
                if gn:
                    hi, lo = coef_hi_lo(ids, cxo, cyo)
                    cb_arr[:, 0:6, jj * 128 : jj * 128 + gn] = hi
                    cb_arr[:, 6:12, jj * 128 : jj * 128 + gn] = lo
                    colors_arr[:gn, jj * 3 : jj * 3 + 3] = colors[ids].astype(
                        np.float16
                    )
                jj += 1
        cb_arr[:, 0:6, X : X + F] = basis6
        cb_arr[:, 6:12, X : X + F] = basis6
        in_maps.append({"cb": cb_arr, "colors": colors_arr})

    import time as _time

    global _last_in_maps
    _last_in_maps = in_maps
    run = _get_runner(nc)
    if _bench_calls:
        return run.time_loop(in_maps, _bench_calls)
    t0 = _time.time()
    results = run(in_maps, reuse_inputs=_time_exec)
    exec_wall = _time.time() - t0

    out = np.zeros((H, W, 3), np.float32)
    hw3 = TILE_COLS // 2
    for core in range(N_CORES):
        o = results[core]["out"]  # [99, n_rounds*F] f16; rows 32i..32i+2 real
        for k in range(n_slots):
            ty, tx = tile_pos[assign[core][k]]
            r, i = divmod(k, GROUP_TILES)
            raw = o[32 * i : 32 * i + 3, r * F : (r + 1) * F].astype(np.float32)
            # columns are ordered [left 16x16 row-major | right 16x16]
            blk = np.empty((3, TILE_ROWS, TILE_COLS), np.float32)
            blk[:, :, 0:hw3] = raw[:, 0 : F // 2].reshape(3, TILE_ROWS, hw3)
            blk[:, :, hw3:] = raw[:, F // 2 : F].reshape(3, TILE_ROWS, hw3)
            out[ty : ty + TILE_ROWS, tx : tx + TILE_COLS, :] = blk.transpose(1, 2, 0)
    if _repeat > 1:
        out /= np.float32(_repeat)
    if _time_exec:
        return out, exec_wall
    return out


# revision 37
# speedup vs baseline: 1.0703x; 1.0703x over previous
"""2D Gaussian splat rasterizer on 8 Trainium2 NeuronCores.

Strategy: shard the image into 128 tiles of 16x32 px (F=512), dealt to 8
cores (16 slots each). Gaussians are culled host-side per tile by the
EXACT min-Mahalanobis-over-rect test (<= raster_ratio^2) and packed into
chunks of <=128 (partition dim). Per chunk, on device:

    arg   = coefT.T @ basis      TensorE, K=12 fp16, 1 cy/row: the 6-term
                                 pixel-basis [x2, xy, y2, x, y, 1] quadratic
                                 (tile-local coords) with each coefficient
                                 split hi/lo across two fp16 rows (full-f32
                                 effective precision; basis values are all
                                 exactly representable in fp16), constant
                                 row folds in ln(opacity)
    alpha = Exp(arg)             ScalarE from PSUM, fp16 out, fused over
                                 FUSE consecutive chunks (no bias, no mask:
                                 the cutoff is applied by the exact cull,
                                 in-tile tails are within tolerance)
    out  += colors.T @ alpha     TensorE, K=128 fp16, PSUM accumulate;
                                 4 tiles share one PSUM bank via
                                 tile_position column groups {0,32,64,96}

Per round of 4 tiles one DVE copy moves the PSUM bank to fp16 SBUF; one
final DMA (partition-strided AP) writes all 16 tiles out. 3 DMAs total.
The full [H, W, 3] image is reassembled host-side (no collectives).
"""

import numpy as np
import concourse.bacc as bacc
import concourse.tile as tile
from concourse import mybir
from concourse.bass_utils import run_bass_kernel_spmd

_runner_cache = {}


def _get_runner(nc):
    """Persistent jitted SPMD executor for a compiled Bass program (modeled on
    bass2jax.run_bass_via_pjrt's multi-core path, but cached so repeat calls
    reuse the same XLA executable — no retrace, no NEFF reload)."""
    key = id(nc)
    if key in _runner_cache:
        return _runner_cache[key]
    import jax
    import jax.numpy as jnp
    from jax.sharding import Mesh, PartitionSpec
    from jax.experimental.shard_map import shard_map
    from concourse import bass2jax, mybir as mb

    bass2jax.install_neuronx_cc_hook()

    in_names, out_names, out_avals, zero_outs = [], [], [], []
    partition_name = nc.partition_id_tensor.name if nc.partition_id_tensor else None
    for alloc in nc.m.functions[0].allocations:
        if not isinstance(alloc, mb.MemoryLocationSet):
            continue
        name = alloc.memorylocations[0].name
        if alloc.kind == "ExternalInput":
            if name != partition_name:
                in_names.append(name)
        elif alloc.kind == "ExternalOutput":
            shape = tuple(alloc.tensor_shape)
            dtype = mb.dt.np(alloc.dtype)
            out_names.append(name)
            out_avals.append(jax.core.ShapedArray(shape, dtype))
            zero_outs.append(np.zeros(shape, dtype))
    n_params = len(in_names)
    all_in = in_names + out_names + ([partition_name] if partition_name else [])

    def _body(*args):
        operands = list(args)
        if partition_name is not None:
            operands.append(bass2jax.partition_id_tensor())
        outs = bass2jax._bass_exec_p.bind(
            *operands,
            out_avals=tuple(out_avals),
            in_names=tuple(all_in),
            out_names=tuple(out_names),
            lowering_input_output_aliases=(),
            sim_require_finite=True,
            sim_require_nnan=True,
            nc=nc,
        )
        return tuple(outs)

    devices = jax.devices()[:N_CORES]
    mesh = Mesh(np.asarray(devices), ("core",))
    in_specs = (PartitionSpec("core"),) * (n_params + len(out_names))
    out_specs = (PartitionSpec("core"),) * len(out_names)
    sharded = jax.jit(
        shard_map(
            _body, mesh=mesh, in_specs=in_specs, out_specs=out_specs, check_rep=False
        ),
        donate_argnums=tuple(range(n_params, n_params + len(out_names))),
        keep_unused=True,
    )

    dev_in_cache = {}

    def run(in_maps, reuse_inputs=False):
        if reuse_inputs and "in" in dev_in_cache:
            concat_in = dev_in_cache["in"]
        else:
            concat_in = [
                np.concatenate([np.asarray(m[nm]) for m in in_maps], axis=0)
                for nm in in_names
            ]
            if reuse_inputs:
                from jax.sharding import NamedSharding

                sh = NamedSharding(mesh, PartitionSpec("core"))
                concat_in = [jax.device_put(a, sh) for a in concat_in]
                for a in concat_in:
                    a.block_until_ready()
                dev_in_cache["in"] = concat_in
        concat_zeros = [
            np.zeros((N_CORES * z.shape[0], *z.shape[1:]), z.dtype) for z in zero_outs
        ]
        out_arrs = sharded(*concat_in, *concat_zeros)
        out_arrs = [a.block_until_ready() for a in out_arrs]
        return [
            {
                nm: np.asarray(out_arrs[i]).reshape(N_CORES, *out_avals[i].shape)[c]
                for i, nm in enumerate(out_names)
            }
            for c in range(N_CORES)
        ]

    def time_loop(in_maps, n_calls):
        """Per-call wall times with inputs and donated zero-outputs pre-staged
        on device; outputs stay on device (only block_until_ready)."""
        import time as _t
        from jax.sharding import NamedSharding

        sh = NamedSharding(mesh, PartitionSpec("core"))
        concat_in = [
            jax.device_put(
                np.concatenate([np.asarray(m[nm]) for m in in_maps], axis=0), sh
            )
            for nm in in_names
        ]
        zeros_sets = [
            [
                jax.device_put(
                    np.zeros((N_CORES * z.shape[0], *z.shape[1:]), z.dtype), sh
                )
                for z in zero_outs
            ]
            for _ in range(n_calls)
        ]
        for a in concat_in:
            a.block_until_ready()
        for zs in zeros_sets:
            for a in zs:
                a.block_until_ready()
        # warm once (executable load)
        outs = sharded(*concat_in, *zeros_sets[0])
        [a.block_until_ready() for a in outs]
        times = []
        for i in range(1, n_calls):
            t0 = _t.perf_counter()
            outs = sharded(*concat_in, *zeros_sets[i])
            [a.block_until_ready() for a in outs]
            times.append(_t.perf_counter() - t0)
        return times

    def stage(in_maps, n_calls):
        """Pre-stage inputs + n_calls sets of donated zeros; return a closure
        that executes once per call (device exec + block)."""
        from jax.sharding import NamedSharding

        sh = NamedSharding(mesh, PartitionSpec("core"))
        concat_in = [
            jax.device_put(
                np.concatenate([np.asarray(m[nm]) for m in in_maps], axis=0), sh
            )
            for nm in in_names
        ]
        zeros_sets = [
            [
                jax.device_put(
                    np.zeros((N_CORES * z.shape[0], *z.shape[1:]), z.dtype), sh
                )
                for z in zero_outs
            ]
            for _ in range(n_calls)
        ]
        for a in concat_in:
            a.block_until_ready()
        for zs in zeros_sets:
            for a in zs:
                a.block_until_ready()
        state = {"i": 0}

        def call():
            i = state["i"]
            state["i"] += 1
            outs = sharded(*concat_in, *zeros_sets[i])
            # force full materialization — under the axon proxy,
            # block_until_ready alone does not wait for device execution
            return [np.asarray(a) for a in outs]

        return call

    def stage_async(in_maps, n_calls):
        """Like stage() but returns call(block=False) that does not wait."""
        from jax.sharding import NamedSharding

        sh = NamedSharding(mesh, PartitionSpec("core"))
        concat_in = [
            jax.device_put(
                np.concatenate([np.asarray(m[nm]) for m in in_maps], axis=0), sh
            )
            for nm in in_names
        ]
        zeros_sets = [
            [
                jax.device_put(
                    np.zeros((N_CORES * z.shape[0], *z.shape[1:]), z.dtype), sh
                )
                for z in zero_outs
            ]
            for _ in range(n_calls)
        ]
        for a in concat_in:
            a.block_until_ready()
        for zs in zeros_sets:
            for a in zs:
                a.block_until_ready()
        state = {"i": 0}

        def call(block=False):
            i = state["i"]
            state["i"] += 1
            outs = sharded(*concat_in, *zeros_sets[i])
            if block:
                outs = [np.asarray(a) for a in outs]
            return outs

        return call

    run.time_loop = time_loop
    run.stage = stage
    run.stage_async = stage_async
    _runner_cache[key] = run
    return run


N_CORES = 8
K = 12  # 6 basis terms x (hi, lo) coefficient rows
TILE_ROWS = 16
TILE_COLS = 32
F = TILE_ROWS * TILE_COLS  # 512 pixels per tile
QUADS = 1  # PE row-group rotation for arg matmul weight loads
FUSE = 3  # chunks per fused Exp activation
GROUP_TILES = 4  # tiles sharing one PSUM out bank via column groups

_prog_cache = {}


FUSE_COLS = 3 * F  # fused-group PSUM capacity (3 banks)


def _build_program(slot_profiles, cutoff, repeat=1):
    """One SPMD program: 16 slots (tiles) per core; slot_profiles[s] is a
    tuple of (c0, w) chunk descriptors — each chunk covers basis/pixel
    columns [c0, c0+w) of its tile and up to 128 gaussians.

    cutoff is unused (kept for cache-key/test harness compatibility)."""
    n_slots = len(slot_profiles)
    n_rounds = n_slots // GROUP_TILES
    tot = sum(len(p) for p in slot_profiles)
    X = tot * 128  # coef columns
    CB = X + F  # coef | basis columns
    nc = bacc.Bacc(
        "TRN2",
        target_bir_lowering=False,
        debug=False,
        enable_asserts=True,
        num_devices=N_CORES,
    )
    f32, f16 = mybir.dt.float32, mybir.dt.float16
    cb_ext = nc.dram_tensor("cb", [QUADS, K, CB], f16, kind="ExternalInput").ap()
    colors_ext = nc.dram_tensor("colors", [128, tot * 3], f16, kind="ExternalInput").ap()
    # 99 partition lines: 4 tile groups at partition offsets {0,32,64,96},
    # rows 3..31 of each group are don't-care (host reads rows 32*i..32*i+2)
    out_ext = nc.dram_tensor("out", [99, n_rounds * F], f16,
                             kind="ExternalOutput").ap()

    # per-round chunk lists: (slot, c0, w, first, last, global idx)
    j = 0
    round_flat = []
    for r in range(n_rounds):
        fl = []
        for s in range(r * GROUP_TILES, (r + 1) * GROUP_TILES):
            prof = slot_profiles[s]
            seen = {}
            for (c0, w) in prof:
                seen[c0] = seen.get(c0, 0) + 1
            done = {}
            for (c0, w) in prof:
                done[c0] = done.get(c0, 0) + 1
                fl.append((s, c0, w, done[c0] == 1, done[c0] == seen[c0], j))
                j += 1
        round_flat.append(fl)

    # greedy fused groups per round: pack chunks into <=FUSE_COLS of PSUM,
    # 512-wide chunks only at 512-aligned offsets (PSUM bank writes)
    def make_groups(fl):
        groups = []
        cur, off = [], 0
        for ch in fl:
            w = ch[2]
            # close when over capacity or when the PSUM write would cross a
            # 512-f32 bank boundary
            if off + w > FUSE_COLS or (off % F) + w > F:
                groups.append(cur)
                cur, off = [], 0
            cur.append((ch, off))
            off += w
        if cur:
            groups.append(cur)
        return groups

    with tile.TileContext(nc) as tc:
        with (
            tc.tile_pool(name="consts", bufs=1) as consts,
            tc.tile_pool(name="work", bufs=3) as work,
            tc.tile_pool(name="outsb", bufs=1) as outsb,
            tc.tile_pool(name="psum_arg", bufs=2, space="PSUM") as psum_arg,
            tc.tile_pool(name="psum_out", bufs=2, space="PSUM") as psum_out,
        ):
            cb_sb = consts.tile([128, CB], f16, name="cb_sb")
            for q in range(QUADS):
                nc.sync.dma_start(
                    out=cb_sb[32 * q : 32 * q + K, :], in_=cb_ext[q]
                )
            colors_sb = consts.tile([128, tot * 3], f16, name="colors_sb")
            nc.sync.dma_start(out=colors_sb[:], in_=colors_ext[:])
            out_sb = outsb.tile([128, n_rounds * F], f16, name="out_sb")

            # preload the Exp activation table while the input DMAs are in
            # flight (the table load rides on this first tiny activation)
            warm_sb = consts.tile([1, 8], f16, name="warm_sb")
            nc.vector.memset(warm_sb[:], 0.0)
            nc.scalar.activation(
                warm_sb[:], warm_sb[:], mybir.ActivationFunctionType.Exp
            )

            for r in range(n_rounds):
                out_ps = psum_out.tile([99, F], f32, tag="out", name=f"out_ps_{r}")
                groups = make_groups(round_flat[r])
                for rep in range(repeat):
                    for g in groups:
                        used = sum(ch[2] for ch, _ in g)
                        arg_ps = psum_arg.tile([128, FUSE_COLS], f32, tag="arg")
                        for (s, c0, w, first, last, jj), off in g:
                            p0 = 32 * (jj % QUADS)
                            nc.tensor.matmul(
                                arg_ps[:, off : off + w],
                                lhsT=cb_sb[p0 : p0 + K, jj * 128 : (jj + 1) * 128],
                                rhs=cb_sb[p0 : p0 + K, X + c0 : X + c0 + w],
                                start=True,
                                stop=True,
                                tile_position=(p0, 0),
                            )
                        alpha_sb = work.tile([128, FUSE_COLS], f16, tag="alpha")
                        nc.scalar.activation(
                            alpha_sb[:, 0:used],
                            arg_ps[:, 0:used],
                            mybir.ActivationFunctionType.Exp,
                        )
                        for (s, c0, w, first, last, jj), off in g:
                            p0 = 32 * (s % GROUP_TILES)
                            nc.tensor.matmul(
                                out_ps[p0 : p0 + 3, c0 : c0 + w],
                                lhsT=colors_sb[:, jj * 3 : (jj + 1) * 3],
                                rhs=alpha_sb[:, off : off + w],
                                start=(first and rep == 0),
                                stop=(last and rep == repeat - 1),
                                tile_position=(0, p0),
                            )
                nc.vector.tensor_copy(out_sb[0:99, r * F : (r + 1) * F], out_ps[:])
                nc.sync.dma_start(
                    out=out_ext[:, r * F : (r + 1) * F],
                    in_=out_sb[0:99, r * F : (r + 1) * F],
                )
    nc.compile()
    return nc


def _get_program(slot_nch, cutoff, repeat=1):
    key = (tuple(slot_nch), float(cutoff), repeat)
    if key not in _prog_cache:
        _prog_cache[key] = _build_program(slot_nch, cutoff, repeat)
    return _prog_cache[key]


NQ = 4  # quarter blocks per tile (each 16 rows x 8 cols = 128 px)
QW = TILE_COLS // NQ  # 8 pixel columns per quarter


def _basis():
    """[6, F] f64 basis terms in tile-local coords (all fp16-exact).

    Pixel-column order: 4 blocks of 128 px (16 rows x 8 cols, row-major per
    block), so quarter-width chunks stream contiguous column ranges."""
    xs = np.arange(TILE_COLS, dtype=np.float64) + 0.5 - TILE_COLS / 2
    ys = np.arange(TILE_ROWS, dtype=np.float64) + 0.5 - TILE_ROWS / 2
    xl = np.empty(F)
    yl = np.empty(F)
    for b in range(NQ):
        for row in range(TILE_ROWS):
            for col in range(QW):
                p = b * (F // NQ) + row * QW + col
                xl[p] = xs[b * QW + col]
                yl[p] = ys[row]
    return np.stack([xl * xl, xl * yl, yl * yl, xl, yl, np.ones_like(xl)], axis=0)


def kernel(
    opacity,
    means,
    stds,
    rhos,
    colors,
    image_height,
    image_width,
    scale_factor,
    raster_ratio,
    _repeat=1,
    _time_exec=False,
    _bench_calls=0,
):
    H = int(image_height)
    W = int(image_width)
    sf = float(scale_factor)
    rr = float(raster_ratio)
    opacity = np.asarray(opacity, np.float64)
    means = np.asarray(means, np.float64)
    stds = np.asarray(stds, np.float64) * sf
    rhos = np.asarray(rhos, np.float64)
    colors = np.asarray(colors, np.float32)
    N = opacity.shape[0]

    n_ty = H // TILE_ROWS
    n_tx = W // TILE_COLS
    n_tiles = n_ty * n_tx
    assert n_tiles % N_CORES == 0
    n_slots = n_tiles // N_CORES
    assert n_slots % GROUP_TILES == 0

    # --- per-gaussian inverse covariance (f64)
    sx, sy = stds[:, 0], stds[:, 1]
    om = 1.0 - rhos * rhos
    ia = 1.0 / (sx * sx * om)
    ib = -rhos / (sx * sy * om)
    ic = 1.0 / (sy * sy * om)
    mx, my = means[:, 0], means[:, 1]
    lnop = np.log(np.maximum(opacity, 1e-30))

    # --- exact ellipse-vs-rect cull: min Mahalanobis^2 over pixel centers
    cut2 = rr * rr + 1e-6

    def min_m2(x0, x1, y0, y1):
        dx0, dx1 = x0 - mx, x1 - mx
        dy0, dy1 = y0 - my, y1 - my
        inside = (dx0 <= 0) & (dx1 >= 0) & (dy0 <= 0) & (dy1 >= 0)
        best = np.full(N, np.inf)
        for cdx in (dx0, dx1):
            dy = np.clip(-ib * cdx / ic, dy0, dy1)
            best = np.minimum(best, ia * cdx * cdx + 2 * ib * cdx * dy + ic * dy * dy)
        for cdy in (dy0, dy1):
            dx = np.clip(-ib * cdy / ia, dx0, dx1)
            best = np.minimum(best, ia * dx * dx + 2 * ib * cdy * dx + ic * cdy * cdy)
        return np.where(inside, 0.0, best)

    tile_ids = []  # full-tile cull
    tile_ids_q = []  # per-quarter culls (NQ lists per tile)
    tile_pos = []
    for tyi in range(n_ty):
        ty = tyi * TILE_ROWS
        y0, y1 = ty + 0.5, ty + TILE_ROWS - 0.5
        for txi in range(n_tx):
            tx = txi * TILE_COLS
            m2 = min_m2(tx + 0.5, tx + TILE_COLS - 0.5, y0, y1)
            tile_ids.append(np.nonzero(m2 <= cut2)[0])
            qs = []
            for b in range(NQ):
                m2q = min_m2(tx + b * QW + 0.5, tx + (b + 1) * QW - 0.5, y0, y1)
                qs.append(np.nonzero(m2q <= cut2)[0])
            tile_ids_q.append(qs)
            tile_pos.append((ty, tx))

    # snake-deal tiles to cores by descending chunk need so the SPMD slot
    # capacities (max over cores per slot) hug each core's real need
    nchs = [max(1, (len(ids) + 127) // 128) for ids in tile_ids]
    # secondary key: per-quarter chunk pattern, so overfull tiles with the
    # same shape land in the same slot (tight per-quarter capacity maxima)
    qpat = [
        tuple((len(q) + 127) // 128 for q in tile_ids_q[t]) for t in range(n_tiles)
    ]
    t_order = sorted(range(n_tiles), key=lambda t: (-nchs[t], qpat[t]))
    assign = [[] for _ in range(N_CORES)]
    for i, t in enumerate(t_order):
        rnd, pos = divmod(i, N_CORES)
        core = pos if rnd % 2 == 0 else N_CORES - 1 - pos
        assign[core].append(t)
    # permute slots so round chunk-counts hug multiples of FUSE (heavy slots
    # paired with light slots within a round)
    perm = []
    lo, hi = 0, n_slots - 1
    while lo < hi:
        perm.extend([lo, lo + 1, hi - 1, hi])
        lo += 2
        hi -= 2
    assign = [[a[p] for p in perm] for a in assign]
    # slot profiles: single full-width chunk when every core's tile fits 128
    # gaussians; otherwise half-width chunks with per-half culls
    slot_profiles = []
    for k in range(n_slots):
        tiles_k = [assign[core][k] for core in range(N_CORES)]
        if max(nchs[t] for t in tiles_k) == 1:
            slot_profiles.append(((0, F),))
        else:
            prof = []
            for b in range(NQ):
                ab = max(
                    1,
                    max(
                        (len(tile_ids_q[t][b]) + 127) // 128 for t in tiles_k
                    ),
                )
                prof.extend([(b * (F // NQ), F // NQ)] * ab)
            slot_profiles.append(tuple(prof))
    slot_profiles = tuple(slot_profiles)
    tot = sum(len(p) for p in slot_profiles)
    X = tot * 128

    nc = _get_program(slot_profiles, 0.0, _repeat)

    basis6 = _basis()  # [6, F] f64, fp16-exact values

    def coef_hi_lo(ids, cxo, cyo):
        mxl = mx[ids] - cxo
        myl = my[ids] - cyo
        A, B, C = ia[ids], ib[ids], ic[ids]
        cf = np.stack(
            [
                -0.5 * A,
                -B,
                -0.5 * C,
                A * mxl + B * myl,
                B * mxl + C * myl,
                -0.5 * (A * mxl * mxl + 2 * B * mxl * myl + C * myl * myl)
                + lnop[ids],
            ],
            axis=0,
        )  # [6, n] f64
        hi = cf.astype(np.float16)
        lo = (cf - hi.astype(np.float64)).astype(np.float16)
        return hi, lo

    in_maps = []
    for core in range(N_CORES):
        cb_arr = np.zeros((QUADS, K, X + F), np.float16)
        colors_arr = np.zeros((128, tot * 3), np.float16)
        jj = 0
        for k in range(n_slots):
            t = assign[core][k]
            ty, tx = tile_pos[t]
            cxo = tx + TILE_COLS / 2
            cyo = ty + TILE_ROWS / 2
            used = {}  # c0 -> chunks consumed for that side
            for (c0, w) in slot_profiles[k]:
                if w == F:
                    ids_side = tile_ids[t]
                else:
                    ids_side = tile_ids_q[t][c0 // (F // NQ)]
                c = used.get(c0, 0)
                used[c0] = c + 1
                ids = ids_side[c * 128 : (c + 1) * 128]
                gn = len(ids)
                if gn:
                    hi, lo = coef_hi_lo(ids, cxo, cyo)
                    cb_arr[:, 0:6, jj * 128 : jj * 128 + gn] = hi
                    cb_arr[:, 6:12, jj * 128 : jj * 128 + gn] = lo
                    colors_arr[:gn, jj * 3 : jj * 3 + 3] = colors[ids].astype(
                        np.float16
                    )
                jj += 1
        cb_arr[:, 0:6, X : X + F] = basis6
        cb_arr[:, 6:12, X : X + F] = basis6
        in_maps.append({"cb": cb_arr, "colors": colors_arr})

    import time as _time

    global _last_in_maps
    _last_in_maps = in_maps
    run = _get_runner(nc)
    if _bench_calls:
        return run.time_loop(in_maps, _bench_calls)
    t0 = _time.time()
    results = run(in_maps, reuse_inputs=_time_exec)
    exec_wall = _time.time() - t0

    out = np.zeros((H, W, 3), np.float32)
    for core in range(N_CORES):
        o = results[core]["out"]  # [99, n_rounds*F] f16; rows 32i..32i+2 real
        for k in range(n_slots):
            ty, tx = tile_pos[assign[core][k]]
            r, i = divmod(k, GROUP_TILES)
            raw = o[32 * i : 32 * i + 3, r * F : (r + 1) * F].astype(np.float32)
            # columns ordered in NQ blocks of (TILE_ROWS x QW, row-major)
            blk = np.empty((3, TILE_ROWS, TILE_COLS), np.float32)
            for b in range(NQ):
                blk[:, :, b * QW : (b + 1) * QW] = raw[
                    :, b * (F // NQ) : (b + 1) * (F // NQ)
                ].reshape(3, TILE_ROWS, QW)
            out[ty : ty + TILE_ROWS, tx : tx + TILE_COLS, :] = blk.transpose(1, 2, 0)
    if _repeat > 1:
        out /= np.float32(_repeat)
    if _time_exec:
        return out, exec_wall
    return out
